# revision 2
# baseline (speedup 1.0000x reference)
"""Trainium2 Bass kernel for nn_FLAttention (B=64, D=512, H=8) — v2.

Math (per batch b, head h), with xa = x*sem_w + sem_b:
    d_{q,k} = alpha_k[h]*xa_k - (alpha_q[h]*xa_q + beta_q[h])
    r = min(1/|d|, RMAX) ; m_q = max_k r ; e = exp(r - m_q)
    Z_q = sum_k e ; N_q = sum_k e*xa_k
    out_q = xa_q + sum_h (alpha_v[h]/sqrt(H)) * N_q/Z_q + sum_h beta_v[h]/sqrt(H)

Engine split (all four compute engines busy):
  PE:     d built as a K=6 rank-6 outer sum in bf16 (3-way bf16 splits of
          the host-fp64 q-side and k-side terms -> fp32-accurate d in PSUM).
  DVE:    one custom op NEGABS_RECIP_CM per tile: reads signed d from PSUM,
          computes -(min(1/|d|, RMAX)) via OR-sign-trick + bitwise-NOT
          reciprocal seed + 1 Newton step, clamps with maxx(y, -RMAX)
          (which also flushes d=0 NaNs to -RMAX), and min-accumulates
          -> negm = -max_k r.  Plus batched 4-wide N-reduces for most groups.
  ScalarE: e16 = Exp(-negr + negm) with accum_out -> Z (fp32); also the
          N-reduce (Copy+accum) for a tunable share of tiles.
  GPSIMD: N-mult en = e*xa batched per (j,h) over [128, 4*512].

Sharding: pure data parallel, 8 batches per core across 8 cores.
"""

import math
import numpy as np
from contextlib import ExitStack

B, D, H = 64, 512, 8
NCORES = 8
BPC = B // NCORES      # batches per core = 8
P = 128                # partitions
QT = D // P            # q tiles per batch = 4
SQH = math.sqrt(H)

RMAX = 1.0e8
# 1-Newton reciprocal constants (optimized, max rel err 1.73e-3)
C0R = -0.23549784719944
C1R = 2.001732349395752

_PROGRAMS = {}

# --- knobs ---
# every SREDn-th (j,h) group does its 4 N-reduces on ScalarE (Copy+accum)
# instead of one batched DVE tensor_reduce. 0 disables S entirely.
SRED = 3
# xbs4 source: "bcast" = gpsimd.partition_broadcast of host-repeated row
XB4 = "bcast"
# every GMULV-th (j,h) group's N-mult runs on DVE tensor_tensor instead of
# GPSIMD (load balance). 0 disables.
GMULV = 6


class _nullcm:
    def __init__(self, it):
        self.it = it
    def __enter__(self):
        return None
    def __exit__(self, *a):
        return False


def _patch_act_tables():
    """Pin Exp/Copy/Identity to natural_log_exp_and_others so the table-load
    pass emits one ACT_TABLE_LOAD."""
    import functools
    from concourse import bacc, mybir, hw_specs

    if getattr(bacc, "_act_tables_pinned", False):
        return
    A = mybir.ActivationFunctionType
    pin = {A.Abs, A.Exp, A.Ln, A.Copy, A.Identity, A.MemsetZero}
    orig = hw_specs.get_activation_tables

    @functools.cache
    def patched(arch):
        full = orig(arch)
        out = {}
        for name, funcs in full.items():
            if name == "natural_log_exp_and_others":
                out[name] = set(funcs)
            else:
                out[name] = set(funcs) - pin
        return out

    bacc.get_activation_tables = patched
    bacc._act_tables_pinned = True


def _register_custom_op():
    """NEGABS_RECIP_CM: out = maxx(recip1(d | -0.0), -RMAX), accum=min.

    Src0 = signed d (fp32, PSUM ok). C2(imm2) = -0.0 (OR mask -> -|d|).
    C0/C1 = seed scale / Newton constant. C3 (spilled to in1 [P,1]) = -RMAX.
    out = -(min(1/|d|, RMAX)); accum_out = min(0, min_k out) = -max_k r.
    d == +-0.0 -> bitwise-NOT seed is NaN -> maxx flushes to -RMAX (DVE max
    semantics treat NaN as missing), exactly the eps-clamped behaviour.
    """
    from concourse import dve_ops
    from concourse.dve_spec import (
        C0, C1, C2, C3, AluOp, Bin, Src0, Zero, lower, maxx, minn, Spec,
        _spill_c3_to_src1,
    )
    from concourse.dve_uop import DveOpSpec

    if hasattr(dve_ops, "NEGABS_RECIP_CM"):
        return dve_ops.NEGABS_RECIP_CM

    negabs = Bin(AluOp.BITWISE_OR, Src0, C2)
    seed = Bin(AluOp.BITWISE_NOT, negabs, negabs)
    y0 = seed * C0
    y1 = y0 * (C1 - negabs * y0)
    body = _spill_c3_to_src1(maxx(y1, C3))

    def _ref(in0, in1, c0, c1, c2):
        x = (np.asarray(in0, np.float32).view(np.int32) | np.int32(-(2**31))
             ).view(np.float32)
        n = (~x.view(np.int32)).view(np.float32)
        yy0 = (n * np.float32(c0)).astype(np.float32)
        yy1 = (yy0 * (np.float32(c1) - x * yy0)).astype(np.float32)
        c3col = np.asarray(in1, np.float32).reshape(in1.shape[0], -1)[:, :1]
        out = np.where(np.isnan(yy1), c3col, np.maximum(yy1, c3col))
        out = out.astype(np.float32)
        acc = np.minimum(
            0.0, out.reshape(out.shape[0], -1).min(axis=-1, keepdims=True)
        ).astype(np.float32)
        return out, acc

    spec = Spec(body=body, accum=minn, accum_init=Zero, reference=_ref)
    name = "NEGABS_RECIP_CM"
    opcode = dve_ops._CUSTOM_DVE_ROW_BASE + len(dve_ops.OPS)
    shas = {}
    for ver in ("v3", "v4"):
        u = lower(spec, ver=ver)
        shas[ver] = DveOpSpec(name=name, opcode=opcode, uops=u, rd1_en=True).sha(ver)
    op = dve_ops.DveOp(name, spec, False, shas)
    dve_ops.OPS.append(op)
    dve_ops.CUSTOM_DVE_SPECS[name] = spec
    dve_ops._SUB_OPCODE_FOR_NAME[name] = opcode
    dve_ops.NEGABS_RECIP_CM = op
    return op


def _build_program(reps=1, for_i_iters=None):
    import concourse.bass as bass
    import concourse.tile as tile
    from concourse import bacc, masks, mybir

    _patch_act_tables()
    OPX = _register_custom_op()

    fp32 = mybir.dt.float32
    fp16 = mybir.dt.float16
    bf16 = mybir.dt.bfloat16
    nc = bacc.Bacc("TRN2", target_bir_lowering=False, debug=False)

    lhs_d = nc.dram_tensor("lhs6", [6, BPC * H * QT * P], bf16, kind="ExternalInput").ap()
    rhs_d = nc.dram_tensor("rhs6", [6, BPC * H * D], bf16, kind="ExternalInput").ap()
    xr16_d = nc.dram_tensor("xrow16", [1, BPC * D], fp16, kind="ExternalInput").ap()
    avp_d = nc.dram_tensor("avp", [P, H * QT], fp32, kind="ExternalInput").ap()
    xap_d = nc.dram_tensor("xap", [P, BPC * QT], fp32, kind="ExternalInput").ap()
    out_d = nc.dram_tensor("out", [BPC * QT, P], fp32, kind="ExternalOutput").ap()

    A = mybir.ActivationFunctionType
    ALU = mybir.AluOpType

    with tile.TileContext(nc) as tc, ExitStack() as ctx:
        const = ctx.enter_context(tc.tile_pool(name="const", bufs=1))
        psum = ctx.enter_context(
            tc.tile_pool(name="psum", bufs=6, space=bass.MemorySpace.PSUM)
        )
        psum_out = ctx.enter_context(
            tc.tile_pool(name="psum_out", bufs=1, space=bass.MemorySpace.PSUM)
        )
        work = ctx.enter_context(tc.tile_pool(name="work", bufs=2))
        negr_pool = ctx.enter_context(tc.tile_pool(name="negr", bufs=7))
        negm_pool = ctx.enter_context(tc.tile_pool(name="negm", bufs=4))
        ebig_pool = ctx.enter_context(tc.tile_pool(name="ebig", bufs=7))
        nz = ctx.enter_context(tc.tile_pool(name="nz", bufs=3))

        ident = const.tile([P, P], fp32)
        masks.make_identity(nc, ident[:])
        rmaxneg = const.tile([P, 1], fp32)
        nc.gpsimd.memset(rmaxneg[:], -RMAX)

        lhs6 = const.tile([6, BPC * H * QT * P], bf16)
        nc.gpsimd.dma_start(lhs6[:], lhs_d[:])
        rhs6 = const.tile([6, BPC * H * D], bf16)
        nc.gpsimd.dma_start(rhs6[:], rhs_d[:])
        xr16 = const.tile([1, BPC * D], fp16)
        nc.gpsimd.dma_start(xr16[:], xr16_d[:])
        avp = const.tile([P, H * QT], fp32)
        nc.gpsimd.dma_start(avp[:], avp_d[:])
        xap = const.tile([P, BPC * QT], fp32)
        nc.gpsimd.dma_start(xap[:], xap_d[:])

        outp = const.tile([P, BPC * QT], fp32)

        def emit_reduce(item):
            # delayed N-reduce: keeps the V/S engine programs from blocking
            # on the GP mult of the group emitted just before them
            en, n32_t, h_, on_s = item[:4]
            if on_s:
                for qt in range(QT):
                    zc = h_ * QT + qt
                    nc.scalar.activation(
                        en[:, qt * D : (qt + 1) * D],
                        en[:, qt * D : (qt + 1) * D],
                        A.Copy,
                        accum_out=n32_t[:, zc : zc + 1],
                    )
            else:
                nc.vector.tensor_reduce(
                    n32_t[:, h_ * QT : (h_ + 1) * QT],
                    en[:].rearrange("p (qt d) -> p qt d", qt=QT, d=D),
                    axis=mybir.AxisListType.X,
                    op=ALU.add,
                )

        def emit_combine(j, z32, n32):
            # out_q = xa_q + cbeta + sum_h avp * N/Z
            rz = nz.tile([P, H * QT], fp32)
            nc.vector.reciprocal_approx_fast(rz[:], z32[:])
            ratio = nz.tile([P, H * QT], fp32)
            nc.vector.tensor_mul(ratio[:], n32[:], rz[:])
            scaled = nz.tile([P, H * QT], fp32)
            nc.vector.tensor_mul(scaled[:], ratio[:], avp[:])
            acc = nz.tile([P, QT], fp32)
            nc.vector.tensor_reduce(
                acc[:],
                scaled[:].rearrange("p (h qt) -> p qt h", h=H, qt=QT),
                axis=mybir.AxisListType.X,
                op=ALU.add,
            )
            nc.vector.tensor_add(
                outp[:, j * QT : (j + 1) * QT],
                acc[:],
                xap[:, j * QT : (j + 1) * QT],
            )

        LAG = 4
        gi = 0  # (j,h) group counter for the SRED split
        rep_cm = (
            tc.For_i(0, for_i_iters, 1)
            if for_i_iters is not None
            else _nullcm(range(reps))
        )
        with rep_cm:
         for rep in range(reps if for_i_iters is None else 1):
          pending = []
          combines = []
          for j in range(BPC):
              # xbs4 [P, QT*D] fp16: xa[b,:] repeated 4x along free
              xbs4 = work.tile([P, QT * D], fp16, tag="xbs4")
              for qt in range(QT):
                  nc.gpsimd.partition_broadcast(
                      xbs4[:, qt * D : (qt + 1) * D],
                      xr16[0:1, j * D : (j + 1) * D],
                  )
              z32 = nz.tile([P, H * QT], fp32)
              n32 = nz.tile([P, H * QT], fp32)
              for h in range(H):
                  if h == 2 and combines:
                      while pending and pending[0][4] < j:
                          emit_reduce(pending.pop(0))
                      emit_combine(*combines.pop(0))
                  rc = (j * H + h) * D
                  negm4 = negm_pool.tile([P, QT], fp32, tag="negm4")
                  ebig = ebig_pool.tile([P, QT * D], fp16, tag="ebig")
                  for qt in range(QT):
                      lc = ((j * H + h) * QT + qt) * P
                      zc = h * QT + qt  # h-major
                      dt = psum.tile([P, D], fp32)
                      nc.tensor.matmul(
                          dt[:], lhs6[:, lc : lc + P], rhs6[:, rc : rc + D],
                          start=True, stop=True,
                      )
                      negr = negr_pool.tile([P, D], fp32, tag="negr")
                      nc.vector._custom_dve(
                          OPX,
                          out=negr[:],
                          in0=dt[:],
                          in1=rmaxneg[:],
                          s0=C0R,
                          s1=C1R,
                          imm2=-0.0,
                          accum_out=negm4[:, qt : qt + 1],
                      )
                      nc.scalar.activation(
                          ebig[:, qt * D : (qt + 1) * D],
                          negr[:],
                          A.Exp,
                          bias=negm4[:, qt : qt + 1],
                          scale=-1.0,
                          accum_out=z32[:, zc : zc + 1],
                      )
                  # N: en = e * xa in place (batched over 4 qt tiles)
                  if GMULV and gi % GMULV == GMULV - 1:
                      nc.vector.tensor_mul(ebig[:], ebig[:], xbs4[:])
                  else:
                      nc.gpsimd.tensor_mul(ebig[:], ebig[:], xbs4[:])
                  on_s = bool(SRED) and gi % SRED == 0
                  pending.append((ebig, n32, h, on_s, j))
                  if len(pending) > LAG:
                      emit_reduce(pending.pop(0))
                  gi += 1
              combines.append((j, z32, n32))

          for item in pending:
              emit_reduce(item)
          for c in combines:
              emit_combine(*c)

        outt = psum_out.tile([BPC * QT, P], fp32)
        nc.tensor.transpose(outt[:], outp[:], ident[:])
        outsb = const.tile([BPC * QT, P], fp32)
        nc.vector.tensor_copy(outsb[:], outt[:])
        nc.gpsimd.dma_start(out_d[:], outsb[:])

    nc.compile()
    return nc


def _get_program(reps=1, for_i_iters=None):
    key = (reps, for_i_iters)
    if key not in _PROGRAMS:
        _PROGRAMS[key] = _build_program(reps, for_i_iters)
    return _PROGRAMS[key]


def _split3_bf16(v):
    """3-way bf16 split of float64 array: v ~= h1+h2+h3 to ~2^-24 rel."""
    import ml_dtypes

    bf = ml_dtypes.bfloat16
    h1 = v.astype(bf)
    r1 = v - h1.astype(np.float64)
    h2 = r1.astype(bf)
    r2 = r1 - h2.astype(np.float64)
    h3 = r2.astype(bf)
    return h1, h2, h3


def _make_in_maps(x, alpha_q, alpha_k, alpha_v, beta_q, beta_v, sem_w, sem_b):
    import ml_dtypes

    f = np.float32
    bf = ml_dtypes.bfloat16
    x = np.asarray(x, f)
    aq = np.asarray(alpha_q, f).reshape(H)
    ak = np.asarray(alpha_k, f).reshape(H)
    av = np.asarray(alpha_v, f).reshape(H)
    bq = np.asarray(beta_q, f).reshape(H)
    bv = np.asarray(beta_v, f).reshape(H)
    sw = np.asarray(sem_w, f).reshape(D)
    sb = np.asarray(sem_b, f).reshape(D)

    xa = (x * sw + sb).astype(f)  # [B, D] (fp32, matches reference rounding)
    cbeta = bv.sum() / SQH

    # h-major: col = h*QT + qt
    avp = np.zeros((P, H * QT), f)
    for h in range(H):
        for qt in range(QT):
            avp[:, h * QT + qt] = av[h] / SQH

    xa64 = xa.astype(np.float64)
    aq64 = aq.astype(np.float64)
    ak64 = ak.astype(np.float64)
    bq64 = bq.astype(np.float64)

    in_maps = []
    for c in range(NCORES):
        bs = slice(c * BPC, (c + 1) * BPC)
        xa_c = xa[bs]            # [BPC, D] fp32
        xa_c64 = xa64[bs]        # [BPC, D] fp64

        # lhs6 [6, BPC*H*QT*P]: rows 0-2 = split3(y), rows 3-5 = 1,1,1
        # y[j,h,qt*P+p] = -(aq[h]*xa[j,qt*P+p] + bq[h])
        y = -(aq64[None, :, None] * xa_c64[:, None, :] + bq64[None, :, None])
        yf = y.reshape(BPC, H, QT, P).reshape(-1)
        y1, y2, y3 = _split3_bf16(yf)
        lhs6 = np.empty((6, BPC * H * QT * P), bf)
        lhs6[0] = y1
        lhs6[1] = y2
        lhs6[2] = y3
        lhs6[3] = bf(1.0)
        lhs6[4] = bf(1.0)
        lhs6[5] = bf(1.0)

        # rhs6 [6, BPC*H*D]: rows 0-2 = 1,0,0; rows 3-5 = split3(ak*xa)
        pk = (ak64[None, :, None] * xa_c64[:, None, :]).reshape(-1)
        p1, p2, p3 = _split3_bf16(pk)
        rhs6 = np.empty((6, BPC * H * D), bf)
        rhs6[0] = bf(1.0)
        rhs6[1] = bf(0.0)
        rhs6[2] = bf(0.0)
        rhs6[3] = p1
        rhs6[4] = p2
        rhs6[5] = p3

        xa_pm = xa_c.reshape(BPC, QT, P).transpose(2, 0, 1)  # [P, BPC, QT]
        xap = (xa_pm + cbeta).reshape(P, BPC * QT).astype(f)
        in_maps.append(
            {
                "lhs6": np.ascontiguousarray(lhs6),
                "rhs6": np.ascontiguousarray(rhs6),
                "xrow16": np.ascontiguousarray(
                    xa_c.astype(np.float16).reshape(1, BPC * D)
                ),
                "avp": avp,
                "xap": np.ascontiguousarray(xap),
            }
        )
    return in_maps


def _assemble(results):
    f = np.float32
    out = np.empty((B, D), f)
    for c in range(NCORES):
        o = np.asarray(results[c]["out"], f)  # [BPC*QT, P]
        o = o.reshape(BPC, QT, P).reshape(BPC, D)
        out[c * BPC : (c + 1) * BPC] = o
    return out


def kernel(x, alpha_q, alpha_k, alpha_v, beta_q, beta_v, sem_w, sem_b):
    from concourse.bass_utils import run_bass_kernel_spmd

    in_maps = _make_in_maps(
        x, alpha_q, alpha_k, alpha_v, beta_q, beta_v, sem_w, sem_b
    )
    nc = _get_program()
    res = run_bass_kernel_spmd(nc, in_maps, core_ids=list(range(NCORES)))
    return _assemble(res.results)


def kernel_sim(x, alpha_q, alpha_k, alpha_v, beta_q, beta_v, sem_w, sem_b, core=0):
    """CoreSim (no hardware) single-core check: returns that core's 8 batches."""
    from concourse.bass_interp import CoreSim

    in_maps = _make_in_maps(
        x, alpha_q, alpha_k, alpha_v, beta_q, beta_v, sem_w, sem_b
    )
    nc = _get_program()
    sim = CoreSim(nc, trace=False)
    for name, arr in in_maps[core].items():
        sim.tensor(name)[:] = arr
    sim.simulate(check_with_hw=False)
    o = np.asarray(sim.tensor("out"), np.float32)
    return o.reshape(BPC, QT, P).reshape(BPC, D)


# revision 3
# speedup vs baseline: 1.0511x; 1.0511x over previous
"""Trainium2 Bass kernel for nn_FLAttention (B=64, D=512, H=8) — v2.

Math (per batch b, head h), with xa = x*sem_w + sem_b:
    d_{q,k} = alpha_k[h]*xa_k - (alpha_q[h]*xa_q + beta_q[h])
    r = min(1/|d|, RMAX) ; m_q = max_k r ; e = exp(r - m_q)
    Z_q = sum_k e ; N_q = sum_k e*xa_k
    out_q = xa_q + sum_h (alpha_v[h]/sqrt(H)) * N_q/Z_q + sum_h beta_v[h]/sqrt(H)

Engine split (all four compute engines busy):
  PE:     d built as a K=6 rank-6 outer sum in bf16 (3-way bf16 splits of
          the host-fp64 q-side and k-side terms -> fp32-accurate d in PSUM).
  DVE:    one custom op NEGABS_RECIP_CM per tile: reads signed d from PSUM,
          computes -(min(1/|d|, RMAX)) via OR-sign-trick + bitwise-NOT
          reciprocal seed + 1 Newton step, clamps with maxx(y, -RMAX)
          (which also flushes d=0 NaNs to -RMAX), and min-accumulates
          -> negm = -max_k r.  Plus batched 4-wide N-reduces for most groups.
  ScalarE: e16 = Exp(-negr + negm) with accum_out -> Z (fp32); also the
          N-reduce (Copy+accum) for a tunable share of tiles.
  GPSIMD: N-mult en = e*xa batched per (j,h) over [128, 4*512].

Sharding: pure data parallel, 8 batches per core across 8 cores.
"""

import math
import numpy as np
from contextlib import ExitStack

B, D, H = 64, 512, 8
NCORES = 8
BPC = B // NCORES      # batches per core = 8
P = 128                # partitions
QT = D // P            # q tiles per batch = 4
SQH = math.sqrt(H)

RMAX = 1.0e8
# 1-Newton reciprocal constants (optimized, max rel err 1.73e-3)
C0R = -0.23549784719944
C1R = 2.001732349395752

_PROGRAMS = {}

# --- knobs ---
# every SREDn-th (j,h) group does its 4 N-reduces on ScalarE (Copy+accum)
# instead of one batched DVE tensor_reduce. 0 disables S entirely.
SRED = 3
# xbs4 source: "bcast" = gpsimd.partition_broadcast of host-repeated row
XB4 = "bcast"
# every GMULV-th (j,h) group's N-mult runs on DVE tensor_tensor instead of
# GPSIMD (load balance). 0 disables.
GMULV = 6


class _nullcm:
    def __init__(self, it):
        self.it = it
    def __enter__(self):
        return None
    def __exit__(self, *a):
        return False


def _patch_act_tables():
    """Pin Exp/Copy/Identity to natural_log_exp_and_others so the table-load
    pass emits one ACT_TABLE_LOAD."""
    import functools
    from concourse import bacc, mybir, hw_specs

    if getattr(bacc, "_act_tables_pinned", False):
        return
    A = mybir.ActivationFunctionType
    pin = {A.Abs, A.Exp, A.Ln, A.Copy, A.Identity, A.MemsetZero}
    orig = hw_specs.get_activation_tables

    @functools.cache
    def patched(arch):
        full = orig(arch)
        out = {}
        for name, funcs in full.items():
            if name == "natural_log_exp_and_others":
                out[name] = set(funcs)
            else:
                out[name] = set(funcs) - pin
        return out

    bacc.get_activation_tables = patched
    bacc._act_tables_pinned = True


def _register_custom_op():
    """NEGABS_RECIP_CM: out = maxx(recip1(d | -0.0), -RMAX), accum=min.

    Src0 = signed d (fp32, PSUM ok). C2(imm2) = -0.0 (OR mask -> -|d|).
    C0/C1 = seed scale / Newton constant. C3 (spilled to in1 [P,1]) = -RMAX.
    out = -(min(1/|d|, RMAX)); accum_out = min(0, min_k out) = -max_k r.
    d == +-0.0 -> bitwise-NOT seed is NaN -> maxx flushes to -RMAX (DVE max
    semantics treat NaN as missing), exactly the eps-clamped behaviour.
    """
    from concourse import dve_ops
    from concourse.dve_spec import (
        C0, C1, C2, C3, AluOp, Bin, Src0, Zero, lower, maxx, minn, Spec,
        _spill_c3_to_src1,
    )
    from concourse.dve_uop import DveOpSpec

    if hasattr(dve_ops, "NEGABS_RECIP_CM"):
        return dve_ops.NEGABS_RECIP_CM

    negabs = Bin(AluOp.BITWISE_OR, Src0, C2)
    seed = Bin(AluOp.BITWISE_NOT, negabs, negabs)
    y0 = seed * C0
    y1 = y0 * (C1 - negabs * y0)
    body = _spill_c3_to_src1(maxx(y1, C3))

    def _ref(in0, in1, c0, c1, c2):
        x = (np.asarray(in0, np.float32).view(np.int32) | np.int32(-(2**31))
             ).view(np.float32)
        n = (~x.view(np.int32)).view(np.float32)
        yy0 = (n * np.float32(c0)).astype(np.float32)
        yy1 = (yy0 * (np.float32(c1) - x * yy0)).astype(np.float32)
        c3col = np.asarray(in1, np.float32).reshape(in1.shape[0], -1)[:, :1]
        out = np.where(np.isnan(yy1), c3col, np.maximum(yy1, c3col))
        out = out.astype(np.float32)
        acc = np.minimum(
            0.0, out.reshape(out.shape[0], -1).min(axis=-1, keepdims=True)
        ).astype(np.float32)
        return out, acc

    spec = Spec(body=body, accum=minn, accum_init=Zero, reference=_ref)
    name = "NEGABS_RECIP_CM"
    opcode = dve_ops._CUSTOM_DVE_ROW_BASE + len(dve_ops.OPS)
    shas = {}
    for ver in ("v3", "v4"):
        u = lower(spec, ver=ver)
        shas[ver] = DveOpSpec(name=name, opcode=opcode, uops=u, rd1_en=True).sha(ver)
    op = dve_ops.DveOp(name, spec, False, shas)
    dve_ops.OPS.append(op)
    dve_ops.CUSTOM_DVE_SPECS[name] = spec
    dve_ops._SUB_OPCODE_FOR_NAME[name] = opcode
    dve_ops.NEGABS_RECIP_CM = op
    return op


def _build_program(reps=1, for_i_iters=None):
    import concourse.bass as bass
    import concourse.tile as tile
    from concourse import bacc, masks, mybir

    _patch_act_tables()
    OPX = _register_custom_op()

    fp32 = mybir.dt.float32
    fp16 = mybir.dt.float16
    bf16 = mybir.dt.bfloat16
    nc = bacc.Bacc("TRN2", target_bir_lowering=False, debug=False)

    lhs_d = nc.dram_tensor("lhs6", [6, BPC * H * QT * P], bf16, kind="ExternalInput").ap()
    rhs_d = nc.dram_tensor("rhs6", [6, BPC * H * D], bf16, kind="ExternalInput").ap()
    xr16_d = nc.dram_tensor("xrow16", [1, BPC * D], fp16, kind="ExternalInput").ap()
    avp_d = nc.dram_tensor("avp", [P, H * QT], fp32, kind="ExternalInput").ap()
    xap_d = nc.dram_tensor("xap", [P, BPC * QT], fp32, kind="ExternalInput").ap()
    out_d = nc.dram_tensor("out", [BPC * QT, P], fp32, kind="ExternalOutput").ap()

    A = mybir.ActivationFunctionType
    ALU = mybir.AluOpType

    with tile.TileContext(nc) as tc, ExitStack() as ctx:
        const = ctx.enter_context(tc.tile_pool(name="const", bufs=1))
        psum = ctx.enter_context(
            tc.tile_pool(name="psum", bufs=7, space=bass.MemorySpace.PSUM)
        )
        psum_out = ctx.enter_context(
            tc.tile_pool(name="psum_out", bufs=1, space=bass.MemorySpace.PSUM)
        )
        work = ctx.enter_context(tc.tile_pool(name="work", bufs=2))
        negr_pool = ctx.enter_context(tc.tile_pool(name="negr", bufs=7))
        negm_pool = ctx.enter_context(tc.tile_pool(name="negm", bufs=4))
        ebig_pool = ctx.enter_context(tc.tile_pool(name="ebig", bufs=8))
        nz = ctx.enter_context(tc.tile_pool(name="nz", bufs=3))

        ident = const.tile([P, P], fp32)
        masks.make_identity(nc, ident[:])
        rmaxneg = const.tile([P, 1], fp32)
        nc.gpsimd.memset(rmaxneg[:], -RMAX)

        lhs6 = const.tile([6, BPC * H * QT * P], bf16)
        nc.gpsimd.dma_start(lhs6[:], lhs_d[:])
        rhs6 = const.tile([6, BPC * H * D], bf16)
        nc.gpsimd.dma_start(rhs6[:], rhs_d[:])
        xr16 = const.tile([1, BPC * D], fp16)
        nc.gpsimd.dma_start(xr16[:], xr16_d[:])
        avp = const.tile([P, H * QT], fp32)
        nc.gpsimd.dma_start(avp[:], avp_d[:])
        xap = const.tile([P, BPC * QT], fp32)
        nc.gpsimd.dma_start(xap[:], xap_d[:])

        outp = const.tile([P, BPC * QT], fp32)

        def emit_reduce(item):
            # delayed N-reduce: keeps the V/S engine programs from blocking
            # on the GP mult of the group emitted just before them
            en, n32_t, h_, on_s = item[:4]
            if on_s:
                for qt in range(QT):
                    zc = h_ * QT + qt
                    nc.scalar.activation(
                        en[:, qt * D : (qt + 1) * D],
                        en[:, qt * D : (qt + 1) * D],
                        A.Copy,
                        accum_out=n32_t[:, zc : zc + 1],
                    )
            else:
                nc.vector.tensor_reduce(
                    n32_t[:, h_ * QT : (h_ + 1) * QT],
                    en[:].rearrange("p (qt d) -> p qt d", qt=QT, d=D),
                    axis=mybir.AxisListType.X,
                    op=ALU.add,
                )

        def emit_combine(j, z32, n32):
            # out_q = xa_q + cbeta + sum_h avp * N/Z
            rz = nz.tile([P, H * QT], fp32)
            nc.vector.reciprocal_approx_fast(rz[:], z32[:])
            ratio = nz.tile([P, H * QT], fp32)
            nc.vector.tensor_mul(ratio[:], n32[:], rz[:])
            scaled = nz.tile([P, H * QT], fp32)
            nc.vector.tensor_mul(scaled[:], ratio[:], avp[:])
            acc = nz.tile([P, QT], fp32)
            nc.vector.tensor_reduce(
                acc[:],
                scaled[:].rearrange("p (h qt) -> p qt h", h=H, qt=QT),
                axis=mybir.AxisListType.X,
                op=ALU.add,
            )
            nc.vector.tensor_add(
                outp[:, j * QT : (j + 1) * QT],
                acc[:],
                xap[:, j * QT : (j + 1) * QT],
            )

        LAG = 5
        gi = 0  # (j,h) group counter for the SRED split
        rep_cm = (
            tc.For_i(0, for_i_iters, 1)
            if for_i_iters is not None
            else _nullcm(range(reps))
        )
        with rep_cm:
         for rep in range(reps if for_i_iters is None else 1):
          pending = []
          combines = []
          for j in range(BPC):
              # xbs4 [P, QT*D] fp16: xa[b,:] repeated 4x along free
              xbs4 = work.tile([P, QT * D], fp16, tag="xbs4")
              for qt in range(QT):
                  nc.gpsimd.partition_broadcast(
                      xbs4[:, qt * D : (qt + 1) * D],
                      xr16[0:1, j * D : (j + 1) * D],
                  )
              z32 = nz.tile([P, H * QT], fp32)
              n32 = nz.tile([P, H * QT], fp32)
              for h in range(H):
                  if h == 2 and combines:
                      while pending and pending[0][4] < j:
                          emit_reduce(pending.pop(0))
                      emit_combine(*combines.pop(0))
                  rc = (j * H + h) * D
                  negm4 = negm_pool.tile([P, QT], fp32, tag="negm4")
                  ebig = ebig_pool.tile([P, QT * D], fp16, tag="ebig")
                  for qt in range(QT):
                      lc = ((j * H + h) * QT + qt) * P
                      zc = h * QT + qt  # h-major
                      dt = psum.tile([P, D], fp32)
                      nc.tensor.matmul(
                          dt[:], lhs6[:, lc : lc + P], rhs6[:, rc : rc + D],
                          start=True, stop=True,
                      )
                      negr = negr_pool.tile([P, D], fp32, tag="negr")
                      nc.vector._custom_dve(
                          OPX,
                          out=negr[:],
                          in0=dt[:],
                          in1=rmaxneg[:],
                          s0=C0R,
                          s1=C1R,
                          imm2=-0.0,
                          accum_out=negm4[:, qt : qt + 1],
                      )
                      nc.scalar.activation(
                          ebig[:, qt * D : (qt + 1) * D],
                          negr[:],
                          A.Exp,
                          bias=negm4[:, qt : qt + 1],
                          scale=-1.0,
                          accum_out=z32[:, zc : zc + 1],
                      )
                  # N: en = e * xa in place (batched over 4 qt tiles)
                  if GMULV and gi % GMULV == GMULV - 1:
                      nc.vector.tensor_mul(ebig[:], ebig[:], xbs4[:])
                  else:
                      nc.gpsimd.tensor_mul(ebig[:], ebig[:], xbs4[:])
                  on_s = bool(SRED) and gi % SRED == 0
                  pending.append((ebig, n32, h, on_s, j))
                  if len(pending) > LAG:
                      emit_reduce(pending.pop(0))
                  gi += 1
              combines.append((j, z32, n32))

          for item in pending:
              emit_reduce(item)
          for c in combines:
              emit_combine(*c)

        outt = psum_out.tile([BPC * QT, P], fp32)
        nc.tensor.transpose(outt[:], outp[:], ident[:])
        outsb = const.tile([BPC * QT, P], fp32)
        nc.vector.tensor_copy(outsb[:], outt[:])
        nc.gpsimd.dma_start(out_d[:], outsb[:])

    nc.compile()
    return nc


def _get_program(reps=1, for_i_iters=None):
    key = (reps, for_i_iters)
    if key not in _PROGRAMS:
        _PROGRAMS[key] = _build_program(reps, for_i_iters)
    return _PROGRAMS[key]


def _split3_bf16(v):
    """3-way bf16 split of float64 array: v ~= h1+h2+h3 to ~2^-24 rel."""
    import ml_dtypes

    bf = ml_dtypes.bfloat16
    h1 = v.astype(bf)
    r1 = v - h1.astype(np.float64)
    h2 = r1.astype(bf)
    r2 = r1 - h2.astype(np.float64)
    h3 = r2.astype(bf)
    return h1, h2, h3


def _make_in_maps(x, alpha_q, alpha_k, alpha_v, beta_q, beta_v, sem_w, sem_b):
    import ml_dtypes

    f = np.float32
    bf = ml_dtypes.bfloat16
    x = np.asarray(x, f)
    aq = np.asarray(alpha_q, f).reshape(H)
    ak = np.asarray(alpha_k, f).reshape(H)
    av = np.asarray(alpha_v, f).reshape(H)
    bq = np.asarray(beta_q, f).reshape(H)
    bv = np.asarray(beta_v, f).reshape(H)
    sw = np.asarray(sem_w, f).reshape(D)
    sb = np.asarray(sem_b, f).reshape(D)

    xa = (x * sw + sb).astype(f)  # [B, D] (fp32, matches reference rounding)
    cbeta = bv.sum() / SQH

    # h-major: col = h*QT + qt
    avp = np.zeros((P, H * QT), f)
    for h in range(H):
        for qt in range(QT):
            avp[:, h * QT + qt] = av[h] / SQH

    xa64 = xa.astype(np.float64)
    aq64 = aq.astype(np.float64)
    ak64 = ak.astype(np.float64)
    bq64 = bq.astype(np.float64)

    in_maps = []
    for c in range(NCORES):
        bs = slice(c * BPC, (c + 1) * BPC)
        xa_c = xa[bs]            # [BPC, D] fp32
        xa_c64 = xa64[bs]        # [BPC, D] fp64

        # lhs6 [6, BPC*H*QT*P]: rows 0-2 = split3(y), rows 3-5 = 1,1,1
        # y[j,h,qt*P+p] = -(aq[h]*xa[j,qt*P+p] + bq[h])
        y = -(aq64[None, :, None] * xa_c64[:, None, :] + bq64[None, :, None])
        yf = y.reshape(BPC, H, QT, P).reshape(-1)
        y1, y2, y3 = _split3_bf16(yf)
        lhs6 = np.empty((6, BPC * H * QT * P), bf)
        lhs6[0] = y1
        lhs6[1] = y2
        lhs6[2] = y3
        lhs6[3] = bf(1.0)
        lhs6[4] = bf(1.0)
        lhs6[5] = bf(1.0)

        # rhs6 [6, BPC*H*D]: rows 0-2 = 1,0,0; rows 3-5 = split3(ak*xa)
        pk = (ak64[None, :, None] * xa_c64[:, None, :]).reshape(-1)
        p1, p2, p3 = _split3_bf16(pk)
        rhs6 = np.empty((6, BPC * H * D), bf)
        rhs6[0] = bf(1.0)
        rhs6[1] = bf(0.0)
        rhs6[2] = bf(0.0)
        rhs6[3] = p1
        rhs6[4] = p2
        rhs6[5] = p3

        xa_pm = xa_c.reshape(BPC, QT, P).transpose(2, 0, 1)  # [P, BPC, QT]
        xap = (xa_pm + cbeta).reshape(P, BPC * QT).astype(f)
        in_maps.append(
            {
                "lhs6": np.ascontiguousarray(lhs6),
                "rhs6": np.ascontiguousarray(rhs6),
                "xrow16": np.ascontiguousarray(
                    xa_c.astype(np.float16).reshape(1, BPC * D)
                ),
                "avp": avp,
                "xap": np.ascontiguousarray(xap),
            }
        )
    return in_maps


def _assemble(results):
    f = np.float32
    out = np.empty((B, D), f)
    for c in range(NCORES):
        o = np.asarray(results[c]["out"], f)  # [BPC*QT, P]
        o = o.reshape(BPC, QT, P).reshape(BPC, D)
        out[c * BPC : (c + 1) * BPC] = o
    return out


def kernel(x, alpha_q, alpha_k, alpha_v, beta_q, beta_v, sem_w, sem_b):
    from concourse.bass_utils import run_bass_kernel_spmd

    in_maps = _make_in_maps(
        x, alpha_q, alpha_k, alpha_v, beta_q, beta_v, sem_w, sem_b
    )
    nc = _get_program()
    res = run_bass_kernel_spmd(nc, in_maps, core_ids=list(range(NCORES)))
    return _assemble(res.results)


def kernel_sim(x, alpha_q, alpha_k, alpha_v, beta_q, beta_v, sem_w, sem_b, core=0):
    """CoreSim (no hardware) single-core check: returns that core's 8 batches."""
    from concourse.bass_interp import CoreSim

    in_maps = _make_in_maps(
        x, alpha_q, alpha_k, alpha_v, beta_q, beta_v, sem_w, sem_b
    )
    nc = _get_program()
    sim = CoreSim(nc, trace=False)
    for name, arr in in_maps[core].items():
        sim.tensor(name)[:] = arr
    sim.simulate(check_with_hw=False)
    o = np.asarray(sim.tensor("out"), np.float32)
    return o.reshape(BPC, QT, P).reshape(BPC, D)


# revision 4
# speedup vs baseline: 1.3675x; 1.3010x over previous
"""Trainium2 Bass kernel for nn_FLAttention (B=64, D=512, H=8) — v2.

Math (per batch b, head h), with xa = x*sem_w + sem_b:
    d_{q,k} = alpha_k[h]*xa_k - (alpha_q[h]*xa_q + beta_q[h])
    r = min(1/|d|, RMAX) ; m_q = max_k r ; e = exp(r - m_q)
    Z_q = sum_k e ; N_q = sum_k e*xa_k
    out_q = xa_q + sum_h (alpha_v[h]/sqrt(H)) * N_q/Z_q + sum_h beta_v[h]/sqrt(H)

Engine split (all four compute engines busy):
  PE:     d built as a K=6 rank-6 outer sum in bf16 (3-way bf16 splits of
          the host-fp64 q-side and k-side terms -> fp32-accurate d in PSUM).
  DVE:    one custom op NEGABS_RECIP_CM per tile: reads signed d from PSUM,
          computes -(min(1/|d|, RMAX)) via OR-sign-trick + bitwise-NOT
          reciprocal seed + 1 Newton step, clamps with maxx(y, -RMAX)
          (which also flushes d=0 NaNs to -RMAX), and min-accumulates
          -> negm = -max_k r.  Plus batched 4-wide N-reduces for most groups.
  ScalarE: e16 = Exp(-negr + negm) with accum_out -> Z (fp32); also the
          N-reduce (Copy+accum) for a tunable share of tiles.
  GPSIMD: N-mult en = e*xa batched per (j,h) over [128, 4*512].

Sharding: pure data parallel, 8 batches per core across 8 cores.
"""

import math
import numpy as np
from contextlib import ExitStack

B, D, H = 64, 512, 8
NCORES = 8
BPC = B // NCORES      # batches per core = 8
P = 128                # partitions
QT = D // P            # q tiles per batch = 4
SQH = math.sqrt(H)

RMAX = 1.0e8
# 1-Newton reciprocal constants (optimized, max rel err 1.73e-3)
C0R = -0.23549784719944
C1R = 2.001732349395752

_PROGRAMS = {}

# --- knobs ---
# every SREDn-th (j,h) group does its 4 N-reduces on ScalarE (Copy+accum)
# instead of one batched DVE tensor_reduce. 0 disables S entirely.
SRED = 4
# xbs4 source: "bcast" = gpsimd.partition_broadcast of host-repeated row
XB4 = "bcast"
# every GMULV-th (j,h) group's N-mult runs on DVE tensor_tensor instead of
# GPSIMD (load balance). 0 disables.
GMULV = 0


class _nullcm:
    def __init__(self, it):
        self.it = it
    def __enter__(self):
        return None
    def __exit__(self, *a):
        return False


def _patch_act_tables():
    """Pin Exp/Copy/Identity to natural_log_exp_and_others so the table-load
    pass emits one ACT_TABLE_LOAD."""
    import functools
    from concourse import bacc, mybir, hw_specs

    if getattr(bacc, "_act_tables_pinned", False):
        return
    A = mybir.ActivationFunctionType
    pin = {A.Abs, A.Exp, A.Ln, A.Copy, A.Identity, A.MemsetZero}
    orig = hw_specs.get_activation_tables

    @functools.cache
    def patched(arch):
        full = orig(arch)
        out = {}
        for name, funcs in full.items():
            if name == "natural_log_exp_and_others":
                out[name] = set(funcs)
            else:
                out[name] = set(funcs) - pin
        return out

    bacc.get_activation_tables = patched
    bacc._act_tables_pinned = True


def _register_custom_op():
    """NEGABS_RECIP_CM: out = maxx(recip1(d | -0.0), -RMAX), accum=min.

    Src0 = signed d (fp32, PSUM ok). C2(imm2) = -0.0 (OR mask -> -|d|).
    C0/C1 = seed scale / Newton constant. C3 (spilled to in1 [P,1]) = -RMAX.
    out = -(min(1/|d|, RMAX)); accum_out = min(0, min_k out) = -max_k r.
    d == +-0.0 -> bitwise-NOT seed is NaN -> maxx flushes to -RMAX (DVE max
    semantics treat NaN as missing), exactly the eps-clamped behaviour.
    """
    from concourse import dve_ops
    from concourse.dve_spec import (
        C0, C1, C2, C3, AluOp, Bin, Src0, Zero, lower, maxx, minn, Spec,
        _spill_c3_to_src1,
    )
    from concourse.dve_uop import DveOpSpec

    if hasattr(dve_ops, "NEGABS_RECIP_CM"):
        return dve_ops.NEGABS_RECIP_CM

    negabs = Bin(AluOp.BITWISE_OR, Src0, C2)
    seed = Bin(AluOp.BITWISE_NOT, negabs, negabs)
    y0 = seed * C0
    y1 = y0 * (C1 - negabs * y0)
    body = _spill_c3_to_src1(maxx(y1, C3))

    def _ref(in0, in1, c0, c1, c2):
        x = (np.asarray(in0, np.float32).view(np.int32) | np.int32(-(2**31))
             ).view(np.float32)
        n = (~x.view(np.int32)).view(np.float32)
        yy0 = (n * np.float32(c0)).astype(np.float32)
        yy1 = (yy0 * (np.float32(c1) - x * yy0)).astype(np.float32)
        c3col = np.asarray(in1, np.float32).reshape(in1.shape[0], -1)[:, :1]
        out = np.where(np.isnan(yy1), c3col, np.maximum(yy1, c3col))
        out = out.astype(np.float32)
        acc = np.minimum(
            0.0, out.reshape(out.shape[0], -1).min(axis=-1, keepdims=True)
        ).astype(np.float32)
        return out, acc

    spec = Spec(body=body, accum=minn, accum_init=Zero, reference=_ref)
    name = "NEGABS_RECIP_CM"
    opcode = dve_ops._CUSTOM_DVE_ROW_BASE + len(dve_ops.OPS)
    shas = {}
    for ver in ("v3", "v4"):
        u = lower(spec, ver=ver)
        shas[ver] = DveOpSpec(name=name, opcode=opcode, uops=u, rd1_en=True).sha(ver)
    op = dve_ops.DveOp(name, spec, False, shas)
    dve_ops.OPS.append(op)
    dve_ops.CUSTOM_DVE_SPECS[name] = spec
    dve_ops._SUB_OPCODE_FOR_NAME[name] = opcode
    dve_ops.NEGABS_RECIP_CM = op
    return op


def _build_program(reps=1, for_i_iters=None):
    import concourse.bass as bass
    import concourse.tile as tile
    from concourse import bacc, masks, mybir

    _patch_act_tables()
    OPX = _register_custom_op()

    fp32 = mybir.dt.float32
    fp16 = mybir.dt.float16
    bf16 = mybir.dt.bfloat16
    nc = bacc.Bacc("TRN2", target_bir_lowering=False, debug=False)

    lhs_d = nc.dram_tensor("lhs6", [6, BPC * H * QT * P], bf16, kind="ExternalInput").ap()
    rhs_d = nc.dram_tensor("rhs6", [6, BPC * H * D], bf16, kind="ExternalInput").ap()
    xr16_d = nc.dram_tensor("xrow16", [1, BPC * QT * D], fp16, kind="ExternalInput").ap()
    avp_d = nc.dram_tensor("avp", [P, H * QT], fp32, kind="ExternalInput").ap()
    xap_d = nc.dram_tensor("xap", [P, BPC * QT], fp32, kind="ExternalInput").ap()
    out_d = nc.dram_tensor("out", [BPC * QT, P], fp32, kind="ExternalOutput").ap()

    A = mybir.ActivationFunctionType
    ALU = mybir.AluOpType

    with tile.TileContext(nc) as tc, ExitStack() as ctx:
        const = ctx.enter_context(tc.tile_pool(name="const", bufs=1))
        psum = ctx.enter_context(
            tc.tile_pool(name="psum", bufs=7, space=bass.MemorySpace.PSUM)
        )
        psum_out = ctx.enter_context(
            tc.tile_pool(name="psum_out", bufs=1, space=bass.MemorySpace.PSUM)
        )
        work = ctx.enter_context(tc.tile_pool(name="work", bufs=2))
        xbs_pool = ctx.enter_context(tc.tile_pool(name="xbs", bufs=4))
        negr_pool = ctx.enter_context(tc.tile_pool(name="negr", bufs=7))
        negm_pool = ctx.enter_context(tc.tile_pool(name="negm", bufs=4))
        ebig_pool = ctx.enter_context(tc.tile_pool(name="ebig", bufs=8))
        nz = ctx.enter_context(tc.tile_pool(name="nz", bufs=3))

        ident = const.tile([P, P], fp32)
        masks.make_identity(nc, ident[:])
        rmaxneg = const.tile([P, 1], fp32)
        nc.gpsimd.memset(rmaxneg[:], -RMAX)

        lhs6 = const.tile([6, BPC * H * QT * P], bf16)
        rhs6 = const.tile([6, BPC * H * D], bf16)
        JW_L = H * QT * P
        JW_R = H * D
        for jj in range(BPC):
            (nc.scalar, nc.gpsimd)[jj % 2].dma_start(
                lhs6[:, jj * JW_L : (jj + 1) * JW_L],
                lhs_d[:, jj * JW_L : (jj + 1) * JW_L],
            )
            (nc.gpsimd, nc.scalar)[jj % 2].dma_start(
                rhs6[:, jj * JW_R : (jj + 1) * JW_R],
                rhs_d[:, jj * JW_R : (jj + 1) * JW_R],
            )
        avp = const.tile([P, H * QT], fp32)
        nc.gpsimd.dma_start(avp[:], avp_d[:])
        xap = const.tile([P, BPC * QT], fp32)
        nc.gpsimd.dma_start(xap[:], xap_d[:])

        outp = const.tile([P, BPC * QT], fp32)
        z32g = const.tile([P, BPC * H * QT], fp32)
        n32g = const.tile([P, BPC * H * QT], fp32)

        def emit_reduce(item):
            # delayed N-reduce: keeps the V/S engine programs from blocking
            # on the GP mult of the group emitted just before them
            en, n32_t, h_, on_s = item[:4]
            if on_s:
                for qt in range(QT):
                    zc = h_ * QT + qt
                    nc.scalar.activation(
                        en[:, qt * D : (qt + 1) * D],
                        en[:, qt * D : (qt + 1) * D],
                        A.Copy,
                        accum_out=n32_t[:, zc : zc + 1],
                    )
            else:
                nc.vector.tensor_reduce(
                    n32_t[:, h_ * QT : (h_ + 1) * QT],
                    en[:].rearrange("p (qt d) -> p qt d", qt=QT, d=D),
                    axis=mybir.AxisListType.X,
                    op=ALU.add,
                )

        LAG = 5
        gi = 0  # (j,h) group counter for the SRED split
        rep_cm = (
            tc.For_i(0, for_i_iters, 1)
            if for_i_iters is not None
            else _nullcm(range(reps))
        )
        with rep_cm:
         for rep in range(reps if for_i_iters is None else 1):
          pending = []
          for j in range(BPC):
              # xbs [P, D] fp16 broadcast across partitions by a
              # 0-stride DMA read; the mult reads it 4x via a 0-stride view
              xbs = xbs_pool.tile([P, D], fp16, tag="xbs4")
              nc.gpsimd.dma_start(
                  xbs[:],
                  xr16_d[0:1, j * QT * D : j * QT * D + D]
                  .partition_broadcast(P),
              )
              xbs4 = xbs[:].unsqueeze(1).broadcast_to((P, QT, D))
              z32 = z32g[:, j * H * QT : (j + 1) * H * QT]
              n32 = n32g[:, j * H * QT : (j + 1) * H * QT]
              for h in range(H):
                  rc = (j * H + h) * D
                  negm4 = negm_pool.tile([P, QT], fp32, tag="negm4")
                  ebig = ebig_pool.tile([P, QT * D], fp16, tag="ebig")
                  for qt in range(QT):
                      lc = ((j * H + h) * QT + qt) * P
                      zc = h * QT + qt  # h-major
                      dt = psum.tile([P, D], fp32)
                      nc.tensor.matmul(
                          dt[:], lhs6[:, lc : lc + P], rhs6[:, rc : rc + D],
                          start=True, stop=True,
                      )
                      negr = negr_pool.tile([P, D], fp32, tag="negr")
                      nc.vector._custom_dve(
                          OPX,
                          out=negr[:],
                          in0=dt[:],
                          in1=rmaxneg[:],
                          s0=C0R,
                          s1=C1R,
                          imm2=-0.0,
                          accum_out=negm4[:, qt : qt + 1],
                      )
                      nc.scalar.activation(
                          ebig[:, qt * D : (qt + 1) * D],
                          negr[:],
                          A.Exp,
                          bias=negm4[:, qt : qt + 1],
                          scale=-1.0,
                          accum_out=z32[:, zc : zc + 1],
                      )
                  # N: en = e * xa in place (batched over 4 qt tiles)
                  eview = ebig[:].rearrange("p (qt d) -> p qt d", qt=QT, d=D)
                  last = gi == BPC * H - 1
                  if last or (GMULV and gi % GMULV == GMULV - 1):
                      nc.vector.tensor_mul(eview, eview, xbs4)
                  else:
                      nc.gpsimd.tensor_mul(eview, eview, xbs4)
                  on_s = bool(SRED) and gi % SRED == 0 and not last
                  # (fast tail: last group mult on V, reduce not on S)
                  pending.append((ebig, n32, h, on_s, j))
                  if len(pending) > LAG:
                      emit_reduce(pending.pop(0))
                  gi += 1

          for item in pending:
              emit_reduce(item)
          # batched combine across all j at once:
          # out = xa + cbeta + sum_h avp*(N/Z)
          rzg = nz.tile([P, BPC * H * QT], fp32)
          nc.vector.reciprocal_approx_fast(rzg[:], z32g[:])
          ratiog = nz.tile([P, BPC * H * QT], fp32)
          nc.vector.tensor_mul(ratiog[:], n32g[:], rzg[:])
          scaledg = nz.tile([P, BPC * H * QT], fp32)
          nc.vector.tensor_mul(
              scaledg[:].rearrange("p (j c) -> p j c", j=BPC, c=H * QT),
              ratiog[:].rearrange("p (j c) -> p j c", j=BPC, c=H * QT),
              avp[:].unsqueeze(1).broadcast_to((P, BPC, H * QT)),
          )
          accg = nz.tile([P, BPC * QT], fp32)
          nc.vector.tensor_reduce(
              accg[:],
              scaledg[:].rearrange(
                  "p (j h qt) -> p j qt h", j=BPC, h=H, qt=QT
              ),
              axis=mybir.AxisListType.X,
              op=ALU.add,
          )
          nc.vector.tensor_add(outp[:], accg[:], xap[:])

        outt = psum_out.tile([BPC * QT, P], fp32)
        nc.tensor.transpose(outt[:], outp[:], ident[:])
        outsb = const.tile([BPC * QT, P], fp32)
        nc.vector.tensor_copy(outsb[:], outt[:])
        nc.gpsimd.dma_start(out_d[:], outsb[:])

    nc.compile()
    return nc


def _get_program(reps=1, for_i_iters=None):
    key = (reps, for_i_iters)
    if key not in _PROGRAMS:
        _PROGRAMS[key] = _build_program(reps, for_i_iters)
    return _PROGRAMS[key]


def _split3_bf16(v):
    """3-way bf16 split of float64 array: v ~= h1+h2+h3 to ~2^-24 rel."""
    import ml_dtypes

    bf = ml_dtypes.bfloat16
    h1 = v.astype(bf)
    r1 = v - h1.astype(np.float64)
    h2 = r1.astype(bf)
    r2 = r1 - h2.astype(np.float64)
    h3 = r2.astype(bf)
    return h1, h2, h3


def _make_in_maps(x, alpha_q, alpha_k, alpha_v, beta_q, beta_v, sem_w, sem_b):
    import ml_dtypes

    f = np.float32
    bf = ml_dtypes.bfloat16
    x = np.asarray(x, f)
    aq = np.asarray(alpha_q, f).reshape(H)
    ak = np.asarray(alpha_k, f).reshape(H)
    av = np.asarray(alpha_v, f).reshape(H)
    bq = np.asarray(beta_q, f).reshape(H)
    bv = np.asarray(beta_v, f).reshape(H)
    sw = np.asarray(sem_w, f).reshape(D)
    sb = np.asarray(sem_b, f).reshape(D)

    xa = (x * sw + sb).astype(f)  # [B, D] (fp32, matches reference rounding)
    cbeta = bv.sum() / SQH

    # h-major: col = h*QT + qt
    avp = np.zeros((P, H * QT), f)
    for h in range(H):
        for qt in range(QT):
            avp[:, h * QT + qt] = av[h] / SQH

    xa64 = xa.astype(np.float64)
    aq64 = aq.astype(np.float64)
    ak64 = ak.astype(np.float64)
    bq64 = bq.astype(np.float64)

    in_maps = []
    for c in range(NCORES):
        bs = slice(c * BPC, (c + 1) * BPC)
        xa_c = xa[bs]            # [BPC, D] fp32
        xa_c64 = xa64[bs]        # [BPC, D] fp64

        # lhs6 [6, BPC*H*QT*P]: rows 0-2 = split3(y), rows 3-5 = 1,1,1
        # y[j,h,qt*P+p] = -(aq[h]*xa[j,qt*P+p] + bq[h])
        y = -(aq64[None, :, None] * xa_c64[:, None, :] + bq64[None, :, None])
        yf = y.reshape(BPC, H, QT, P).reshape(-1)
        y1, y2, y3 = _split3_bf16(yf)
        lhs6 = np.empty((6, BPC * H * QT * P), bf)
        lhs6[0] = y1
        lhs6[1] = y2
        lhs6[2] = y3
        lhs6[3] = bf(1.0)
        lhs6[4] = bf(1.0)
        lhs6[5] = bf(1.0)

        # rhs6 [6, BPC*H*D]: rows 0-2 = 1,0,0; rows 3-5 = split3(ak*xa)
        pk = (ak64[None, :, None] * xa_c64[:, None, :]).reshape(-1)
        p1, p2, p3 = _split3_bf16(pk)
        rhs6 = np.empty((6, BPC * H * D), bf)
        rhs6[0] = bf(1.0)
        rhs6[1] = bf(0.0)
        rhs6[2] = bf(0.0)
        rhs6[3] = p1
        rhs6[4] = p2
        rhs6[5] = p3

        xa_pm = xa_c.reshape(BPC, QT, P).transpose(2, 0, 1)  # [P, BPC, QT]
        xap = (xa_pm + cbeta).reshape(P, BPC * QT).astype(f)
        in_maps.append(
            {
                "lhs6": np.ascontiguousarray(lhs6),
                "rhs6": np.ascontiguousarray(rhs6),
                "xrow16": np.ascontiguousarray(
                    np.tile(
                        xa_c.astype(np.float16)[:, None, :], (1, QT, 1)
                    ).reshape(1, BPC * QT * D)
                ),
                "avp": avp,
                "xap": np.ascontiguousarray(xap),
            }
        )
    return in_maps


def _assemble(results):
    f = np.float32
    out = np.empty((B, D), f)
    for c in range(NCORES):
        o = np.asarray(results[c]["out"], f)  # [BPC*QT, P]
        o = o.reshape(BPC, QT, P).reshape(BPC, D)
        out[c * BPC : (c + 1) * BPC] = o
    return out


def kernel(x, alpha_q, alpha_k, alpha_v, beta_q, beta_v, sem_w, sem_b):
    from concourse.bass_utils import run_bass_kernel_spmd

    in_maps = _make_in_maps(
        x, alpha_q, alpha_k, alpha_v, beta_q, beta_v, sem_w, sem_b
    )
    nc = _get_program()
    res = run_bass_kernel_spmd(nc, in_maps, core_ids=list(range(NCORES)))
    return _assemble(res.results)


def kernel_sim(x, alpha_q, alpha_k, alpha_v, beta_q, beta_v, sem_w, sem_b, core=0):
    """CoreSim (no hardware) single-core check: returns that core's 8 batches."""
    from concourse.bass_interp import CoreSim

    in_maps = _make_in_maps(
        x, alpha_q, alpha_k, alpha_v, beta_q, beta_v, sem_w, sem_b
    )
    nc = _get_program()
    sim = CoreSim(nc, trace=False)
    for name, arr in in_maps[core].items():
        sim.tensor(name)[:] = arr
    sim.simulate(check_with_hw=False)
    o = np.asarray(sim.tensor("out"), np.float32)
    return o.reshape(BPC, QT, P).reshape(BPC, D)


# revision 5
# speedup vs baseline: 1.3912x; 1.0174x over previous
"""Trainium2 Bass kernel for nn_FLAttention (B=64, D=512, H=8) — v2.

Math (per batch b, head h), with xa = x*sem_w + sem_b:
    d_{q,k} = alpha_k[h]*xa_k - (alpha_q[h]*xa_q + beta_q[h])
    r = min(1/|d|, RMAX) ; m_q = max_k r ; e = exp(r - m_q)
    Z_q = sum_k e ; N_q = sum_k e*xa_k
    out_q = xa_q + sum_h (alpha_v[h]/sqrt(H)) * N_q/Z_q + sum_h beta_v[h]/sqrt(H)

Engine split (all four compute engines busy):
  PE:     d built as a K=6 rank-6 outer sum in bf16 (3-way bf16 splits of
          the host-fp64 q-side and k-side terms -> fp32-accurate d in PSUM).
  DVE:    one custom op NEGABS_RECIP_CM per tile: reads signed d from PSUM,
          computes -(min(1/|d|, RMAX)) via OR-sign-trick + bitwise-NOT
          reciprocal seed + 1 Newton step, clamps with maxx(y, -RMAX)
          (which also flushes d=0 NaNs to -RMAX), and min-accumulates
          -> negm = -max_k r.  Plus batched 4-wide N-reduces for most groups.
  ScalarE: e16 = Exp(-negr + negm) with accum_out -> Z (fp32); also the
          N-reduce (Copy+accum) for a tunable share of tiles.
  GPSIMD: N-mult en = e*xa batched per (j,h) over [128, 4*512].

Sharding: pure data parallel, 8 batches per core across 8 cores.
"""

import math
import numpy as np
from contextlib import ExitStack

B, D, H = 64, 512, 8
NCORES = 8
BPC = B // NCORES      # batches per core = 8
P = 128                # partitions
QT = D // P            # q tiles per batch = 4
SQH = math.sqrt(H)

RMAX = 1.0e8
# 1-Newton reciprocal constants (optimized, max rel err 1.73e-3)
C0R = -0.23549784719944
C1R = 2.001732349395752

_PROGRAMS = {}

# --- knobs ---
# every SREDn-th (j,h) group does its 4 N-reduces on ScalarE (Copy+accum)
# instead of one batched DVE tensor_reduce. 0 disables S entirely.
SRED = 4
# xbs4 source: "bcast" = gpsimd.partition_broadcast of host-repeated row
XB4 = "bcast"
# every GMULV-th (j,h) group's N-mult runs on DVE tensor_tensor instead of
# GPSIMD (load balance). 0 disables.
GMULV = 0


class _nullcm:
    def __init__(self, it):
        self.it = it
    def __enter__(self):
        return None
    def __exit__(self, *a):
        return False


def _patch_act_tables():
    """Pin Exp/Copy/Identity to natural_log_exp_and_others so the table-load
    pass emits one ACT_TABLE_LOAD."""
    import functools
    from concourse import bacc, mybir, hw_specs

    if getattr(bacc, "_act_tables_pinned", False):
        return
    A = mybir.ActivationFunctionType
    pin = {A.Abs, A.Exp, A.Ln, A.Copy, A.Identity, A.MemsetZero}
    orig = hw_specs.get_activation_tables

    @functools.cache
    def patched(arch):
        full = orig(arch)
        out = {}
        for name, funcs in full.items():
            if name == "natural_log_exp_and_others":
                out[name] = set(funcs)
            else:
                out[name] = set(funcs) - pin
        return out

    bacc.get_activation_tables = patched
    bacc._act_tables_pinned = True


def _register_custom_op():
    """NEGABS_RECIP_CM: out = maxx(recip1(d | -0.0), -RMAX), accum=min.

    Src0 = signed d (fp32, PSUM ok). C2(imm2) = -0.0 (OR mask -> -|d|).
    C0/C1 = seed scale / Newton constant. C3 (spilled to in1 [P,1]) = -RMAX.
    out = -(min(1/|d|, RMAX)); accum_out = min(0, min_k out) = -max_k r.
    d == +-0.0 -> bitwise-NOT seed is NaN -> maxx flushes to -RMAX (DVE max
    semantics treat NaN as missing), exactly the eps-clamped behaviour.
    """
    from concourse import dve_ops
    from concourse.dve_spec import (
        C0, C1, C2, C3, AluOp, Bin, Src0, Zero, lower, maxx, minn, Spec,
        _spill_c3_to_src1,
    )
    from concourse.dve_uop import DveOpSpec

    if hasattr(dve_ops, "NEGABS_RECIP_CM"):
        return dve_ops.NEGABS_RECIP_CM

    negabs = Bin(AluOp.BITWISE_OR, Src0, C2)
    seed = Bin(AluOp.BITWISE_NOT, negabs, negabs)
    y0 = seed * C0
    y1 = y0 * (C1 - negabs * y0)
    body = _spill_c3_to_src1(maxx(y1, C3))

    def _ref(in0, in1, c0, c1, c2):
        x = (np.asarray(in0, np.float32).view(np.int32) | np.int32(-(2**31))
             ).view(np.float32)
        n = (~x.view(np.int32)).view(np.float32)
        yy0 = (n * np.float32(c0)).astype(np.float32)
        yy1 = (yy0 * (np.float32(c1) - x * yy0)).astype(np.float32)
        c3col = np.asarray(in1, np.float32).reshape(in1.shape[0], -1)[:, :1]
        out = np.where(np.isnan(yy1), c3col, np.maximum(yy1, c3col))
        out = out.astype(np.float32)
        acc = np.minimum(
            0.0, out.reshape(out.shape[0], -1).min(axis=-1, keepdims=True)
        ).astype(np.float32)
        return out, acc

    spec = Spec(body=body, accum=minn, accum_init=Zero, reference=_ref)
    name = "NEGABS_RECIP_CM"
    opcode = dve_ops._CUSTOM_DVE_ROW_BASE + len(dve_ops.OPS)
    shas = {}
    for ver in ("v3", "v4"):
        u = lower(spec, ver=ver)
        shas[ver] = DveOpSpec(name=name, opcode=opcode, uops=u, rd1_en=True).sha(ver)
    op = dve_ops.DveOp(name, spec, False, shas)
    dve_ops.OPS.append(op)
    dve_ops.CUSTOM_DVE_SPECS[name] = spec
    dve_ops._SUB_OPCODE_FOR_NAME[name] = opcode
    dve_ops.NEGABS_RECIP_CM = op
    return op


def _build_program(reps=1, for_i_iters=None):
    import concourse.bass as bass
    import concourse.tile as tile
    from concourse import bacc, masks, mybir

    _patch_act_tables()
    OPX = _register_custom_op()

    fp32 = mybir.dt.float32
    fp16 = mybir.dt.float16
    bf16 = mybir.dt.bfloat16
    nc = bacc.Bacc("TRN2", target_bir_lowering=False, debug=False)

    lhs_d = nc.dram_tensor("lhs6", [6, BPC * H * QT * P], bf16, kind="ExternalInput").ap()
    rhs_d = nc.dram_tensor("rhs6", [6, BPC * H * D], bf16, kind="ExternalInput").ap()
    xr16_d = nc.dram_tensor("xrow16", [1, BPC * QT * D], fp16, kind="ExternalInput").ap()
    avp_d = nc.dram_tensor("avp", [P, H * QT], fp32, kind="ExternalInput").ap()
    xap_d = nc.dram_tensor("xap", [P, BPC * QT], fp32, kind="ExternalInput").ap()
    out_d = nc.dram_tensor("out", [BPC * QT, P], fp32, kind="ExternalOutput").ap()

    A = mybir.ActivationFunctionType
    ALU = mybir.AluOpType

    with tile.TileContext(nc) as tc, ExitStack() as ctx:
        const = ctx.enter_context(tc.tile_pool(name="const", bufs=1))
        psum = ctx.enter_context(
            tc.tile_pool(name="psum", bufs=7, space=bass.MemorySpace.PSUM)
        )
        psum_out = ctx.enter_context(
            tc.tile_pool(name="psum_out", bufs=1, space=bass.MemorySpace.PSUM)
        )
        work = ctx.enter_context(tc.tile_pool(name="work", bufs=2))
        negr_pool = ctx.enter_context(tc.tile_pool(name="negr", bufs=7))
        negm_pool = ctx.enter_context(tc.tile_pool(name="negm", bufs=4))
        ebig_pool = ctx.enter_context(tc.tile_pool(name="ebig", bufs=8))
        nz = ctx.enter_context(tc.tile_pool(name="nz", bufs=3))

        ident = const.tile([P, P], fp32)
        masks.make_identity(nc, ident[:])
        rmaxneg = const.tile([P, 1], fp32)
        nc.gpsimd.memset(rmaxneg[:], -RMAX)

        xbs_all = const.tile([P, BPC * D], fp16)

        def xbs_dma(jj, eng=None):
            (eng or nc.gpsimd).dma_start(
                xbs_all[:, jj * D : (jj + 1) * D],
                xr16_d[0:1, jj * QT * D : jj * QT * D + D]
                .partition_broadcast(P),
            )

        lhs6 = const.tile([6, BPC * H * QT * P], bf16)
        rhs6 = const.tile([6, BPC * H * D], bf16)
        JW_L = H * QT * P
        JW_R = H * D
        for jj in range(BPC):
            (nc.scalar, nc.gpsimd)[jj % 2].dma_start(
                lhs6[:, jj * JW_L : (jj + 1) * JW_L],
                lhs_d[:, jj * JW_L : (jj + 1) * JW_L],
            )
            if jj == 0:
                xbs_dma(0, nc.scalar)
                xbs_dma(1, nc.scalar)
            (nc.gpsimd, nc.scalar)[jj % 2].dma_start(
                rhs6[:, jj * JW_R : (jj + 1) * JW_R],
                rhs_d[:, jj * JW_R : (jj + 1) * JW_R],
            )
        for jj in range(2, BPC):
            xbs_dma(jj)
        avp = const.tile([P, H * QT], fp32)
        nc.gpsimd.dma_start(avp[:], avp_d[:])
        xap = const.tile([P, BPC * QT], fp32)
        nc.gpsimd.dma_start(xap[:], xap_d[:])

        outp = const.tile([P, BPC * QT], fp32)
        z32g = const.tile([P, BPC * H * QT], fp32)
        n32g = const.tile([P, BPC * H * QT], fp32)

        def emit_reduce(item):
            # delayed N-reduce: keeps the V/S engine programs from blocking
            # on the GP mult of the group emitted just before them
            en, n32_t, h_, on_s = item[:4]
            if on_s:
                for qt in range(QT):
                    zc = h_ * QT + qt
                    nc.scalar.activation(
                        en[:, qt * D : (qt + 1) * D],
                        en[:, qt * D : (qt + 1) * D],
                        A.Copy,
                        accum_out=n32_t[:, zc : zc + 1],
                    )
            else:
                nc.vector.tensor_reduce(
                    n32_t[:, h_ * QT : (h_ + 1) * QT],
                    en[:].rearrange("p (qt d) -> p qt d", qt=QT, d=D),
                    axis=mybir.AxisListType.X,
                    op=ALU.add,
                )

        LAG = 5
        gi = 0  # (j,h) group counter for the SRED split
        rep_cm = (
            tc.For_i(0, for_i_iters, 1)
            if for_i_iters is not None
            else _nullcm(range(reps))
        )
        with rep_cm:
         for rep in range(reps if for_i_iters is None else 1):
          pending = []
          for j in range(BPC):
              # xbs broadcast across partitions by 0-stride DMA reads
              # issued up front; the mult reads 4x via a 0-stride view
              xbs4 = (
                  xbs_all[:, j * D : (j + 1) * D]
                  .unsqueeze(1)
                  .broadcast_to((P, QT, D))
              )
              z32 = z32g[:, j * H * QT : (j + 1) * H * QT]
              n32 = n32g[:, j * H * QT : (j + 1) * H * QT]
              for h in range(H):
                  rc = (j * H + h) * D
                  negm4 = negm_pool.tile([P, QT], fp32, tag="negm4")
                  ebig = ebig_pool.tile([P, QT * D], fp16, tag="ebig")
                  for qt in range(QT):
                      lc = ((j * H + h) * QT + qt) * P
                      zc = h * QT + qt  # h-major
                      dt = psum.tile([P, D], fp32)
                      nc.tensor.matmul(
                          dt[:], lhs6[:, lc : lc + P], rhs6[:, rc : rc + D],
                          start=True, stop=True,
                      )
                      negr = negr_pool.tile([P, D], fp32, tag="negr")
                      nc.vector._custom_dve(
                          OPX,
                          out=negr[:],
                          in0=dt[:],
                          in1=rmaxneg[:],
                          s0=C0R,
                          s1=C1R,
                          imm2=-0.0,
                          accum_out=negm4[:, qt : qt + 1],
                      )
                      nc.scalar.activation(
                          ebig[:, qt * D : (qt + 1) * D],
                          negr[:],
                          A.Exp,
                          bias=negm4[:, qt : qt + 1],
                          scale=-1.0,
                          accum_out=z32[:, zc : zc + 1],
                      )
                  # N: en = e * xa in place (batched over 4 qt tiles)
                  eview = ebig[:].rearrange("p (qt d) -> p qt d", qt=QT, d=D)
                  last = gi == BPC * H - 1
                  if last or (GMULV and gi % GMULV == GMULV - 1):
                      nc.vector.tensor_mul(eview, eview, xbs4)
                  else:
                      nc.gpsimd.tensor_mul(eview, eview, xbs4)
                  on_s = bool(SRED) and gi % SRED == 0 and not last
                  # (fast tail: last group mult on V, reduce not on S)
                  pending.append((ebig, n32, h, on_s, j))
                  if len(pending) > LAG:
                      emit_reduce(pending.pop(0))
                  gi += 1

          for item in pending:
              emit_reduce(item)
          # batched combine across all j at once:
          # out = xa + cbeta + sum_h avp*(N/Z)
          rzg = nz.tile([P, BPC * H * QT], fp32)
          nc.vector.reciprocal_approx_fast(rzg[:], z32g[:])
          ratiog = nz.tile([P, BPC * H * QT], fp32)
          nc.vector.tensor_mul(ratiog[:], n32g[:], rzg[:])
          scaledg = nz.tile([P, BPC * H * QT], fp32)
          nc.vector.tensor_mul(
              scaledg[:].rearrange("p (j c) -> p j c", j=BPC, c=H * QT),
              ratiog[:].rearrange("p (j c) -> p j c", j=BPC, c=H * QT),
              avp[:].unsqueeze(1).broadcast_to((P, BPC, H * QT)),
          )
          accg = nz.tile([P, BPC * QT], fp32)
          nc.vector.tensor_reduce(
              accg[:],
              scaledg[:].rearrange(
                  "p (j h qt) -> p j qt h", j=BPC, h=H, qt=QT
              ),
              axis=mybir.AxisListType.X,
              op=ALU.add,
          )
          nc.vector.tensor_add(outp[:], accg[:], xap[:])

        outt = psum_out.tile([BPC * QT, P], fp32)
        nc.tensor.transpose(outt[:], outp[:], ident[:])
        outsb = const.tile([BPC * QT, P], fp32)
        nc.vector.tensor_copy(outsb[:], outt[:])
        nc.gpsimd.dma_start(out_d[:], outsb[:])

    nc.compile()
    return nc


def _get_program(reps=1, for_i_iters=None):
    key = (reps, for_i_iters)
    if key not in _PROGRAMS:
        _PROGRAMS[key] = _build_program(reps, for_i_iters)
    return _PROGRAMS[key]


def _split3_bf16(v):
    """3-way bf16 split of float64 array: v ~= h1+h2+h3 to ~2^-24 rel."""
    import ml_dtypes

    bf = ml_dtypes.bfloat16
    h1 = v.astype(bf)
    r1 = v - h1.astype(np.float64)
    h2 = r1.astype(bf)
    r2 = r1 - h2.astype(np.float64)
    h3 = r2.astype(bf)
    return h1, h2, h3


def _make_in_maps(x, alpha_q, alpha_k, alpha_v, beta_q, beta_v, sem_w, sem_b):
    import ml_dtypes

    f = np.float32
    bf = ml_dtypes.bfloat16
    x = np.asarray(x, f)
    aq = np.asarray(alpha_q, f).reshape(H)
    ak = np.asarray(alpha_k, f).reshape(H)
    av = np.asarray(alpha_v, f).reshape(H)
    bq = np.asarray(beta_q, f).reshape(H)
    bv = np.asarray(beta_v, f).reshape(H)
    sw = np.asarray(sem_w, f).reshape(D)
    sb = np.asarray(sem_b, f).reshape(D)

    xa = (x * sw + sb).astype(f)  # [B, D] (fp32, matches reference rounding)
    cbeta = bv.sum() / SQH

    # h-major: col = h*QT + qt
    avp = np.zeros((P, H * QT), f)
    for h in range(H):
        for qt in range(QT):
            avp[:, h * QT + qt] = av[h] / SQH

    xa64 = xa.astype(np.float64)
    aq64 = aq.astype(np.float64)
    ak64 = ak.astype(np.float64)
    bq64 = bq.astype(np.float64)

    in_maps = []
    for c in range(NCORES):
        bs = slice(c * BPC, (c + 1) * BPC)
        xa_c = xa[bs]            # [BPC, D] fp32
        xa_c64 = xa64[bs]        # [BPC, D] fp64

        # lhs6 [6, BPC*H*QT*P]: rows 0-2 = split3(y), rows 3-5 = 1,1,1
        # y[j,h,qt*P+p] = -(aq[h]*xa[j,qt*P+p] + bq[h])
        y = -(aq64[None, :, None] * xa_c64[:, None, :] + bq64[None, :, None])
        yf = y.reshape(BPC, H, QT, P).reshape(-1)
        y1, y2, y3 = _split3_bf16(yf)
        lhs6 = np.empty((6, BPC * H * QT * P), bf)
        lhs6[0] = y1
        lhs6[1] = y2
        lhs6[2] = y3
        lhs6[3] = bf(1.0)
        lhs6[4] = bf(1.0)
        lhs6[5] = bf(1.0)

        # rhs6 [6, BPC*H*D]: rows 0-2 = 1,0,0; rows 3-5 = split3(ak*xa)
        pk = (ak64[None, :, None] * xa_c64[:, None, :]).reshape(-1)
        p1, p2, p3 = _split3_bf16(pk)
        rhs6 = np.empty((6, BPC * H * D), bf)
        rhs6[0] = bf(1.0)
        rhs6[1] = bf(0.0)
        rhs6[2] = bf(0.0)
        rhs6[3] = p1
        rhs6[4] = p2
        rhs6[5] = p3

        xa_pm = xa_c.reshape(BPC, QT, P).transpose(2, 0, 1)  # [P, BPC, QT]
        xap = (xa_pm + cbeta).reshape(P, BPC * QT).astype(f)
        in_maps.append(
            {
                "lhs6": np.ascontiguousarray(lhs6),
                "rhs6": np.ascontiguousarray(rhs6),
                "xrow16": np.ascontiguousarray(
                    np.tile(
                        xa_c.astype(np.float16)[:, None, :], (1, QT, 1)
                    ).reshape(1, BPC * QT * D)
                ),
                "avp": avp,
                "xap": np.ascontiguousarray(xap),
            }
        )
    return in_maps


def _assemble(results):
    f = np.float32
    out = np.empty((B, D), f)
    for c in range(NCORES):
        o = np.asarray(results[c]["out"], f)  # [BPC*QT, P]
        o = o.reshape(BPC, QT, P).reshape(BPC, D)
        out[c * BPC : (c + 1) * BPC] = o
    return out


def kernel(x, alpha_q, alpha_k, alpha_v, beta_q, beta_v, sem_w, sem_b):
    from concourse.bass_utils import run_bass_kernel_spmd

    in_maps = _make_in_maps(
        x, alpha_q, alpha_k, alpha_v, beta_q, beta_v, sem_w, sem_b
    )
    nc = _get_program()
    res = run_bass_kernel_spmd(nc, in_maps, core_ids=list(range(NCORES)))
    return _assemble(res.results)


def kernel_sim(x, alpha_q, alpha_k, alpha_v, beta_q, beta_v, sem_w, sem_b, core=0):
    """CoreSim (no hardware) single-core check: returns that core's 8 batches."""
    from concourse.bass_interp import CoreSim

    in_maps = _make_in_maps(
        x, alpha_q, alpha_k, alpha_v, beta_q, beta_v, sem_w, sem_b
    )
    nc = _get_program()
    sim = CoreSim(nc, trace=False)
    for name, arr in in_maps[core].items():
        sim.tensor(name)[:] = arr
    sim.simulate(check_with_hw=False)
    o = np.asarray(sim.tensor("out"), np.float32)
    return o.reshape(BPC, QT, P).reshape(BPC, D)
